# revision 23
# baseline (speedup 1.0000x reference)
"""Trainium2 Bass kernel for nn_ClusterClassifier (moe_routing).

Strategy: expert-parallel with host-side token routing.
  - Host groups tokens by cluster label; core i gets cluster i's tokens
    (gathered + transposed to [H, T]) and cluster i's decoder weight
    (pre-transposed to [H, vocab_i], LN gamma folded in, zero-padded).
  - Device (per core): h = xT.T @ W_t^T  -> erf-GELU -> LayerNorm
    (bn_stats + Newton-refined rsqrt) -> PE transpose -> logits =
    hn @ Wd'^T (+ folded bias) -> DMA out [T_cap, V_cap] fp32.
  - Host scatters the compact per-cluster logits into zero-filled full
    outputs (the mask semantics of the reference).

All matmuls run in float32r (TF32-class fp32 fast path, ~1e-4 rel err).
"""

import numpy as np

H = 768
KT = H // 128  # 6 contraction chunks
N_CLUSTERS = 8
VOCABS = [3000, 4000, 5000, 6000, 3000, 4000, 5000, 6000]
LN_EPS = 1e-12
VTILE = 512

_prog_cache: dict = {}
DIAG_SKIP_OUT_DMA = False   # diagnostic: skip out DMAs (wrong results)
DIAG_SKIP_OUT_COPY = False  # diagnostic: skip psum->sbuf copies + out DMAs
DEC_BF16 = False  # decode matmul in bf16 (faster, ~10x looser numerics)
PRE_ROUND = True  # host-round inputs to f32r, load via HWDGE (no cast DMA)


def _round_f32r(a: np.ndarray) -> np.ndarray:
    """Round fp32 to float32r (11-bit mantissa, RNE) — matches the HW
    casting-DMA rounding measured on TRN2."""
    b = np.ascontiguousarray(a, dtype=np.float32).view(np.uint32)
    lsb = (b >> 12) & np.uint32(1)
    r = (b + np.uint32(0x7FF) + lsb) & np.uint32(0xFFFFF000)
    return r.view(np.float32)


def _build_program(T_cap: int, V_cap: int, with_bt: bool, with_bd: bool,
                   use_gelu: bool = True, reps: int = 1, dec_bf16: bool = False,
                   pre_rounded: bool = False):
    import concourse.bacc as bacc
    import concourse.mybir as mybir
    import concourse.tile as tile

    f32 = mybir.dt.float32
    f32r = mybir.dt.float32r
    AF = mybir.ActivationFunctionType
    ALU = mybir.AluOpType

    NT = T_cap // 128
    NV = V_cap // VTILE

    nc = bacc.Bacc("TRN2", target_bir_lowering=False, debug=False, num_devices=8)

    _in_dt_early = mybir.dt.float32r if pre_rounded else f32
    xT_d = nc.dram_tensor("xT", [H, T_cap], _in_dt_early, kind="ExternalInput").ap()
    wtT_d = nc.dram_tensor("wtT", [H, H], _in_dt_early, kind="ExternalInput").ap()
    bf16 = mybir.dt.bfloat16
    in_dt = f32r if pre_rounded else f32
    wd_in_dt = bf16 if dec_bf16 else in_dt
    wdT_d = nc.dram_tensor("wdT", [H, V_cap], wd_in_dt, kind="ExternalInput").ap()
    id_d = nc.dram_tensor("ident", [128, 128], _in_dt_early, kind="ExternalInput").ap()
    ones_d = nc.dram_tensor("ones1", [1, 128], _in_dt_early, kind="ExternalInput").ap()
    if with_bt:
        bt_d = nc.dram_tensor("btT", [1, H], _in_dt_early, kind="ExternalInput").ap()
    if with_bd:
        bd_d = nc.dram_tensor("bdT", [1, V_cap], _in_dt_early, kind="ExternalInput").ap()
    out_d = nc.dram_tensor("out", [T_cap, V_cap], f32, kind="ExternalOutput").ap()

    out_tiled = out_d.rearrange("(nt p) (nv n) -> nt p nv n", p=128, n=VTILE)

    with tile.TileContext(nc) as tc:
        with (
            tc.tile_pool(name="consts", bufs=1) as consts,
            tc.tile_pool(name="hnt", bufs=1) as hnt_pool,
            tc.tile_pool(name="wd", bufs=3) as wd_pool,
            tc.tile_pool(name="work", bufs=2) as work,
            tc.tile_pool(name="stats", bufs=4) as stats,
            tc.tile_pool(name="outp", bufs=6) as outp,
            tc.tile_pool(name="hps", bufs=2, space="PSUM") as hps_pool,
            tc.tile_pool(name="tpps", bufs=1, space="PSUM") as tpps_pool,
            tc.tile_pool(name="decps", bufs=3, space="PSUM") as decps_pool,
        ):
            # ---- constants / full-kernel-lifetime tensors ----
            _cdma = nc.sync if pre_rounded else nc.gpsimd
            ident_sb = consts.tile([128, 128], f32r)
            _cdma.dma_start(out=ident_sb, in_=id_d)
            ones_sb = consts.tile([1, 128], f32r)
            _cdma.dma_start(out=ones_sb, in_=ones_d)
            eps_sb = consts.tile([128, 1], f32)
            nc.vector.memset(eps_sb, LN_EPS)
            if with_bt:
                btT_sb = consts.tile([1, H], f32r)
                _cdma.dma_start(out=btT_sb, in_=bt_d)
            if with_bd:
                bdT_sb = consts.tile([1, V_cap], f32r)
                _cdma.dma_start(out=bdT_sb, in_=bd_d)

            for _rep in range(reps):
                run_body(nc, tc, consts, hnt_pool, wd_pool, work, stats, outp,
                         hps_pool, tpps_pool, decps_pool, xT_d, wtT_d, wdT_d,
                         out_tiled, ident_sb, ones_sb, eps_sb,
                         btT_sb if with_bt else None,
                         bdT_sb if with_bd else None,
                         T_cap, V_cap, use_gelu, dec_bf16, pre_rounded)

    nc.compile()
    return nc


def run_body(nc, tc, consts, hnt_pool, wd_pool, work, stats, outp,
             hps_pool, tpps_pool, decps_pool, xT_d, wtT_d, wdT_d,
             out_tiled, ident_sb, ones_sb, eps_sb, btT_sb, bdT_sb,
             T_cap, V_cap, use_gelu, dec_bf16=False, pre_rounded=False):
    import concourse.mybir as mybir
    f32 = mybir.dt.float32
    f32r = mybir.dt.float32r
    dec_dt = mybir.dt.bfloat16 if dec_bf16 else f32r
    in_dma = nc.sync if pre_rounded else nc.gpsimd
    AF = mybir.ActivationFunctionType
    ALU = mybir.AluOpType
    NT = T_cap // 128
    NV = V_cap // VTILE
    with_bt = btT_sb is not None
    with_bd = bdT_sb is not None

    # per-k tiles so the first transform matmul only waits on ~1 MB of DMA
    xT_k = xT_d.rearrange("(c p) t -> c p t", p=128)
    wtT_k = wtT_d.rearrange("(c p) o -> c p o", p=128)
    xT_sb, wtT_sb = [], []
    for k in range(KT):
        xk = consts.tile([128, T_cap], f32r, tag=f"xT{k}", name=f"xT{k}")
        in_dma.dma_start(out=xk, in_=xT_k[k])
        xT_sb.append(xk)
        wk = consts.tile([128, H], f32r, tag=f"wtT{k}", name=f"wtT{k}")
        in_dma.dma_start(out=wk, in_=wtT_k[k])
        wtT_sb.append(wk)

    if True:
        if True:
            # one tile per token-tile so decode(v, t) depends only on its own
            # transposed slice, not on all of phase 1
            hnT_t = [hnt_pool.tile([128, KT, 128], dec_dt, tag=f"hnT{t}", name=f"hnT{t}")
                     for t in range(T_cap // 128)]

            # ---- phase 1: transform + layernorm + transpose ----
            for t in range(NT):
                h_ps = hps_pool.tile([128, H], f32)
                for lo, hi in ((0, 512), (512, 768)):
                    for k in range(KT):
                        nc.tensor.matmul(
                            h_ps[:, lo:hi],
                            xT_sb[k][:, t * 128:(t + 1) * 128],
                            wtT_sb[k][:, lo:hi],
                            start=(k == 0),
                            stop=(k == KT - 1 and not with_bt),
                        )
                    if with_bt:
                        nc.tensor.matmul(
                            h_ps[:, lo:hi], ones_sb, btT_sb[0:1, lo:hi],
                            start=False, stop=True,
                        )
                # GELU (erf variant) PSUM -> SBUF
                hg = work.tile([128, H], f32, tag="hg")
                nc.scalar.activation(hg, h_ps, AF.Gelu if use_gelu else AF.Identity)
                # mean/var via bn_stats (3 subgroups of 256)
                st = stats.tile([128, 3, 6], f32, tag="bnst")
                for g in range(3):
                    nc.vector.bn_stats(out=st[:, g, :], in_=hg[:, g * 256:(g + 1) * 256])
                mv = stats.tile([128, 2], f32, tag="mv")
                nc.vector.bn_aggr(out=mv, in_=st)
                # rstd = 1/sqrt(var+eps), one Newton step for the Sqrt LUT
                sd = stats.tile([128, 1], f32, tag="sd")
                nc.scalar.activation(sd, mv[:, 1:2], AF.Sqrt, bias=eps_sb)
                r0 = stats.tile([128, 1], f32, tag="r0")
                nc.vector.reciprocal(r0, sd)
                vpe = stats.tile([128, 1], f32, tag="vpe")
                nc.vector.tensor_scalar_add(vpe, mv[:, 1:2], LN_EPS)
                t1 = stats.tile([128, 1], f32, tag="t1")
                # t1 = (r0*r0)*vpe ; t1 = t1*-0.5 + 1.5 ; rstd = t1*r0
                nc.vector.tensor_scalar(t1, r0, r0, vpe, ALU.mult, ALU.mult)
                nc.vector.tensor_scalar(t1, t1, -0.5, 1.5, ALU.mult, ALU.add)
                rstd = stats.tile([128, 1], f32, tag="rstd")
                nc.vector.tensor_mul(rstd, t1, r0)
                nmu = stats.tile([128, 1], f32, tag="nmu")
                # nmu = (mv0 * rstd) * -1
                nc.vector.tensor_scalar(nmu, mv[:, 0:1], rstd, -1.0, ALU.mult, ALU.mult)
                # hn = (hg - mu) * rstd, rounded to f32r
                hn = work.tile([128, H], f32r, tag="hn")
                nc.scalar.activation(hn, hg, AF.Identity, bias=nmu, scale=rstd)
                # transpose into hnT [k, t]
                for k in range(KT):
                    tp = tpps_pool.tile([128, 128], f32r)
                    nc.tensor.transpose(tp, hn[:, k * 128:(k + 1) * 128], ident_sb)
                    nc.scalar.copy(out=hnT_t[t][:, k, :], in_=tp)

            # ---- phase 2: decode ----
            for v in range(NV):
                wd_sb = wd_pool.tile([128, KT, VTILE], dec_dt, tag="wd")
                wd_src = wdT_d[:, v * VTILE:(v + 1) * VTILE].rearrange(
                    "(c p) n -> p c n", p=128)
                if dec_bf16 or pre_rounded:
                    nc.sync.dma_start(out=wd_sb, in_=wd_src)
                else:
                    nc.gpsimd.dma_start(out=wd_sb, in_=wd_src)
                for t in range(NT):
                    ps = decps_pool.tile([128, VTILE], f32)
                    for k in range(KT):
                        nc.tensor.matmul(
                            ps,
                            hnT_t[t][:, k, :],
                            wd_sb[:, k, :],
                            start=(k == 0),
                            stop=(k == KT - 1 and not with_bd),
                        )
                    if with_bd:
                        nc.tensor.matmul(
                            ps, ones_sb, bdT_sb[0:1, v * VTILE:(v + 1) * VTILE],
                            start=False, stop=True,
                        )
                    if DIAG_SKIP_OUT_COPY:
                        continue
                    ot = outp.tile([128, VTILE], f32, tag="ot")
                    if (v * NT + t) % 2 == 0:
                        nc.scalar.copy(out=ot, in_=ps)
                    else:
                        nc.vector.tensor_copy(out=ot, in_=ps)
                    if not DIAG_SKIP_OUT_DMA:
                        nc.sync.dma_start(out=out_tiled[t, :, v, :], in_=ot)


def _get_program(T_cap, V_cap, with_bt, with_bd):
    key = (T_cap, V_cap, with_bt, with_bd, DEC_BF16, PRE_ROUND)
    if key not in _prog_cache:
        _prog_cache[key] = _build_program(T_cap, V_cap, with_bt, with_bd,
                                          dec_bf16=DEC_BF16,
                                          pre_rounded=PRE_ROUND)
    return _prog_cache[key]


def kernel(last_hidden_states, cluster_labels, W_t, b_t, ln_gamma, ln_beta,
           dec_weights, dec_biases):
    from concourse import bass_utils

    x = np.ascontiguousarray(np.asarray(last_hidden_states, dtype=np.float32))
    B, S, Hx = x.shape
    assert Hx == H
    labels = np.asarray(cluster_labels).reshape(-1)
    W_t = np.asarray(W_t, dtype=np.float32)
    b_t = np.asarray(b_t, dtype=np.float32)
    gamma = np.asarray(ln_gamma, dtype=np.float32)
    beta = np.asarray(ln_beta, dtype=np.float32)

    x_flat = x.reshape(-1, H)
    idxs = [np.nonzero(labels == i)[0] for i in range(N_CLUSTERS)]
    counts = [len(ix) for ix in idxs]
    T_cap = max(128, ((max(counts) + 127) // 128) * 128)
    V_cap = max(((v + VTILE - 1) // VTILE) * VTILE for v in VOCABS)

    with_bt = bool(np.any(b_t != 0.0))
    wtT = np.ascontiguousarray(W_t.T)

    # fold LN affine into decoder weights/biases:
    # (hn*gamma+beta) @ Wd^T + b == hn @ (Wd*gamma)^T + (b + Wd@beta)
    wdTs, bds = [], []
    for i in range(N_CLUSTERS):
        wd = np.asarray(dec_weights[i], dtype=np.float32)
        bd = np.asarray(dec_biases[i], dtype=np.float32)
        v = wd.shape[0]
        wdT = np.zeros((H, V_cap), dtype=np.float32)
        wdT[:, :v] = (wd * gamma[None, :]).T
        bfold = np.zeros((V_cap,), dtype=np.float32)
        bfold[:v] = bd + wd @ beta
        if DEC_BF16:
            import ml_dtypes
            wdT = wdT.astype(ml_dtypes.bfloat16)
        wdTs.append(wdT)
        bds.append(bfold)
    with_bd = bool(any(np.any(b != 0.0) for b in bds))

    nc = _get_program(T_cap, V_cap, with_bt, with_bd)

    ident = np.eye(128, dtype=np.float32)
    ones1 = np.ones((1, 128), dtype=np.float32)

    if PRE_ROUND:
        wtT = _round_f32r(wtT)
        if not DEC_BF16:
            wdTs = [_round_f32r(w) for w in wdTs]

    in_maps = []
    for i in range(N_CLUSTERS):
        xT = np.zeros((H, T_cap), dtype=np.float32)
        if counts[i]:
            xT[:, :counts[i]] = x_flat[idxs[i]].T
        if PRE_ROUND:
            xT = _round_f32r(xT)
        m = {"xT": xT, "wtT": wtT, "wdT": wdTs[i], "ident": ident, "ones1": ones1}
        if with_bt:
            m["btT"] = _round_f32r(b_t.reshape(1, H)) if PRE_ROUND else b_t.reshape(1, H)
        if with_bd:
            m["bdT"] = _round_f32r(bds[i].reshape(1, V_cap)) if PRE_ROUND else bds[i].reshape(1, V_cap)
        in_maps.append(m)

    res = bass_utils.run_bass_kernel_spmd(nc, in_maps, core_ids=list(range(8)))

    outs = []
    for i in range(N_CLUSTERS):
        v = VOCABS[i]
        full = np.zeros((B * S, v), dtype=np.float32)
        if counts[i]:
            full[idxs[i]] = res.results[i]["out"][:counts[i], :v]
        outs.append(full.reshape(B, S, v))
    return tuple(outs)


# revision 24
# speedup vs baseline: 1.1173x; 1.1173x over previous
"""Trainium2 Bass kernel for nn_ClusterClassifier (moe_routing).

Strategy: expert-parallel with host-side token routing.
  - Host groups tokens by cluster label; core i gets cluster i's tokens
    (gathered + transposed to [H, T]) and cluster i's decoder weight
    (pre-transposed to [H, vocab_i], LN gamma folded in, zero-padded).
  - Device (per core): h = xT.T @ W_t^T  -> erf-GELU -> LayerNorm
    (bn_stats + Newton-refined rsqrt) -> PE transpose -> logits =
    hn @ Wd'^T (+ folded bias) -> DMA out [T_cap, V_cap] fp32.
  - Host scatters the compact per-cluster logits into zero-filled full
    outputs (the mask semantics of the reference).

All matmuls run in float32r (TF32-class fp32 fast path, ~1e-4 rel err).
"""

import numpy as np

H = 768
KT = H // 128  # 6 contraction chunks
N_CLUSTERS = 8
VOCABS = [3000, 4000, 5000, 6000, 3000, 4000, 5000, 6000]
LN_EPS = 1e-12
VTILE = 512

_prog_cache: dict = {}
DIAG_SKIP_OUT_DMA = False   # diagnostic: skip out DMAs (wrong results)
DIAG_SKIP_OUT_COPY = False  # diagnostic: skip psum->sbuf copies + out DMAs
DEC_BF16 = False  # decode matmul in bf16 (faster, ~10x looser numerics)
PRE_ROUND = True  # host-round inputs to f32r, load via HWDGE (no cast DMA)


def _round_f32r(a: np.ndarray) -> np.ndarray:
    """Round fp32 to float32r (11-bit mantissa, RNE) — matches the HW
    casting-DMA rounding measured on TRN2."""
    b = np.ascontiguousarray(a, dtype=np.float32).view(np.uint32)
    lsb = (b >> 12) & np.uint32(1)
    r = (b + np.uint32(0x7FF) + lsb) & np.uint32(0xFFFFF000)
    return r.view(np.float32)


def _build_program(T_cap: int, V_cap: int, with_bt: bool, with_bd: bool,
                   use_gelu: bool = True, reps: int = 1, dec_bf16: bool = False,
                   pre_rounded: bool = False):
    import concourse.bacc as bacc
    import concourse.mybir as mybir
    import concourse.tile as tile

    f32 = mybir.dt.float32
    f32r = mybir.dt.float32r
    AF = mybir.ActivationFunctionType
    ALU = mybir.AluOpType

    NT = T_cap // 128
    NV = V_cap // VTILE

    nc = bacc.Bacc("TRN2", target_bir_lowering=False, debug=False, num_devices=8)

    _in_dt_early = mybir.dt.float32r if pre_rounded else f32
    xT_d = nc.dram_tensor("xT", [H, T_cap], _in_dt_early, kind="ExternalInput").ap()
    wtT_d = nc.dram_tensor("wtT", [H, H], _in_dt_early, kind="ExternalInput").ap()
    bf16 = mybir.dt.bfloat16
    in_dt = f32r if pre_rounded else f32
    wd_in_dt = bf16 if dec_bf16 else in_dt
    wdT_d = nc.dram_tensor("wdT", [H, V_cap], wd_in_dt, kind="ExternalInput").ap()
    id_d = nc.dram_tensor("ident", [128, 128], _in_dt_early, kind="ExternalInput").ap()
    ones_d = nc.dram_tensor("ones1", [1, 128], _in_dt_early, kind="ExternalInput").ap()
    if with_bt:
        bt_d = nc.dram_tensor("btT", [1, H], _in_dt_early, kind="ExternalInput").ap()
    if with_bd:
        bd_d = nc.dram_tensor("bdT", [1, V_cap], _in_dt_early, kind="ExternalInput").ap()
    out_d = nc.dram_tensor("out", [T_cap, V_cap], f32, kind="ExternalOutput").ap()

    out_tiled = out_d.rearrange("(nt p) (nv n) -> nt p nv n", p=128, n=VTILE)

    with tile.TileContext(nc) as tc:
        with (
            tc.tile_pool(name="consts", bufs=1) as consts,
            tc.tile_pool(name="hnt", bufs=1) as hnt_pool,
            tc.tile_pool(name="wd", bufs=2) as wd_pool,
            tc.tile_pool(name="work", bufs=2) as work,
            tc.tile_pool(name="stats", bufs=4) as stats,
            tc.tile_pool(name="outp", bufs=6) as outp,
            tc.tile_pool(name="hps", bufs=2, space="PSUM") as hps_pool,
            tc.tile_pool(name="tpps", bufs=1, space="PSUM") as tpps_pool,
            tc.tile_pool(name="decps", bufs=1, space="PSUM") as decps_pool,
        ):
            # ---- constants / full-kernel-lifetime tensors ----
            _cdma = nc.sync if pre_rounded else nc.gpsimd
            ident_sb = consts.tile([128, 128], f32r)
            _cdma.dma_start(out=ident_sb, in_=id_d)
            ones_sb = consts.tile([1, 128], f32r)
            _cdma.dma_start(out=ones_sb, in_=ones_d)
            eps_sb = consts.tile([128, 1], f32)
            nc.vector.memset(eps_sb, LN_EPS)
            if with_bt:
                btT_sb = consts.tile([1, H], f32r)
                _cdma.dma_start(out=btT_sb, in_=bt_d)
            if with_bd:
                bdT_sb = consts.tile([1, V_cap], f32r)
                _cdma.dma_start(out=bdT_sb, in_=bd_d)

            for _rep in range(reps):
                run_body(nc, tc, consts, hnt_pool, wd_pool, work, stats, outp,
                         hps_pool, tpps_pool, decps_pool, xT_d, wtT_d, wdT_d,
                         out_tiled, ident_sb, ones_sb, eps_sb,
                         btT_sb if with_bt else None,
                         bdT_sb if with_bd else None,
                         T_cap, V_cap, use_gelu, dec_bf16, pre_rounded)

    nc.compile()
    return nc


def run_body(nc, tc, consts, hnt_pool, wd_pool, work, stats, outp,
             hps_pool, tpps_pool, decps_pool, xT_d, wtT_d, wdT_d,
             out_tiled, ident_sb, ones_sb, eps_sb, btT_sb, bdT_sb,
             T_cap, V_cap, use_gelu, dec_bf16=False, pre_rounded=False):
    import concourse.mybir as mybir
    f32 = mybir.dt.float32
    f32r = mybir.dt.float32r
    dec_dt = mybir.dt.bfloat16 if dec_bf16 else f32r
    in_dma = nc.sync if pre_rounded else nc.gpsimd
    AF = mybir.ActivationFunctionType
    ALU = mybir.AluOpType
    NT = T_cap // 128
    NV = V_cap // VTILE
    with_bt = btT_sb is not None
    with_bd = bdT_sb is not None

    # per-k tiles so the first transform matmul only waits on ~1 MB of DMA
    xT_k = xT_d.rearrange("(c p) t -> c p t", p=128)
    wtT_k = wtT_d.rearrange("(c p) o -> c p o", p=128)
    xT_sb, wtT_sb = [], []
    for k in range(KT):
        xk = consts.tile([128, T_cap], f32r, tag=f"xT{k}", name=f"xT{k}")
        in_dma.dma_start(out=xk, in_=xT_k[k])
        xT_sb.append(xk)
        wk = consts.tile([128, H], f32r, tag=f"wtT{k}", name=f"wtT{k}")
        in_dma.dma_start(out=wk, in_=wtT_k[k])
        wtT_sb.append(wk)

    if True:
        if True:
            # one tile per token-tile so decode(v, t) depends only on its own
            # transposed slice, not on all of phase 1
            hnT_t = [hnt_pool.tile([128, KT, 128], dec_dt, tag=f"hnT{t}", name=f"hnT{t}")
                     for t in range(T_cap // 128)]

            # ---- phase 1: transform + layernorm + transpose ----
            for t in range(NT):
                h_ps = hps_pool.tile([128, H], f32)
                for lo, hi in ((0, 512), (512, 768)):
                    for k in range(KT):
                        nc.tensor.matmul(
                            h_ps[:, lo:hi],
                            xT_sb[k][:, t * 128:(t + 1) * 128],
                            wtT_sb[k][:, lo:hi],
                            start=(k == 0),
                            stop=(k == KT - 1 and not with_bt),
                        )
                    if with_bt:
                        nc.tensor.matmul(
                            h_ps[:, lo:hi], ones_sb, btT_sb[0:1, lo:hi],
                            start=False, stop=True,
                        )
                # GELU (erf variant) PSUM -> SBUF
                hg = work.tile([128, H], f32, tag="hg")
                nc.scalar.activation(hg, h_ps, AF.Gelu if use_gelu else AF.Identity)
                # mean/var via bn_stats (3 subgroups of 256)
                st = stats.tile([128, 3, 6], f32, tag="bnst")
                for g in range(3):
                    nc.vector.bn_stats(out=st[:, g, :], in_=hg[:, g * 256:(g + 1) * 256])
                mv = stats.tile([128, 2], f32, tag="mv")
                nc.vector.bn_aggr(out=mv, in_=st)
                # rstd = 1/sqrt(var+eps), one Newton step for the Sqrt LUT
                sd = stats.tile([128, 1], f32, tag="sd")
                nc.scalar.activation(sd, mv[:, 1:2], AF.Sqrt, bias=eps_sb)
                r0 = stats.tile([128, 1], f32, tag="r0")
                nc.vector.reciprocal(r0, sd)
                vpe = stats.tile([128, 1], f32, tag="vpe")
                nc.vector.tensor_scalar_add(vpe, mv[:, 1:2], LN_EPS)
                t1 = stats.tile([128, 1], f32, tag="t1")
                # t1 = (r0*r0)*vpe ; t1 = t1*-0.5 + 1.5 ; rstd = t1*r0
                nc.vector.tensor_scalar(t1, r0, r0, vpe, ALU.mult, ALU.mult)
                nc.vector.tensor_scalar(t1, t1, -0.5, 1.5, ALU.mult, ALU.add)
                rstd = stats.tile([128, 1], f32, tag="rstd")
                nc.vector.tensor_mul(rstd, t1, r0)
                nmu = stats.tile([128, 1], f32, tag="nmu")
                # nmu = (mv0 * rstd) * -1
                nc.vector.tensor_scalar(nmu, mv[:, 0:1], rstd, -1.0, ALU.mult, ALU.mult)
                # hn = (hg - mu) * rstd, rounded to f32r
                hn = work.tile([128, H], f32r, tag="hn")
                nc.scalar.activation(hn, hg, AF.Identity, bias=nmu, scale=rstd)
                # transpose into hnT [k, t]
                for k in range(KT):
                    tp = tpps_pool.tile([128, 128], f32r)
                    nc.tensor.transpose(tp, hn[:, k * 128:(k + 1) * 128], ident_sb)
                    nc.scalar.copy(out=hnT_t[t][:, k, :], in_=tp)

            # ---- phase 2: decode ----
            # vocab-groups of G: consecutive matmuls share the same stationary
            # hnT[t][k] across the group's v-tiles (weight-load reuse on PE)
            G = 3
            for vg in range(NV // G):
                wd3 = []
                for j in range(G):
                    v = vg * G + j
                    wd_sb = wd_pool.tile([128, KT, VTILE], dec_dt,
                                         tag=f"wd{j}", name=f"wd{vg}_{j}")
                    wd_src = wdT_d[:, v * VTILE:(v + 1) * VTILE].rearrange(
                        "(c p) n -> p c n", p=128)
                    if dec_bf16 or pre_rounded:
                        nc.sync.dma_start(out=wd_sb, in_=wd_src)
                    else:
                        nc.gpsimd.dma_start(out=wd_sb, in_=wd_src)
                    wd3.append(wd_sb)
                for t in range(NT):
                    ps3 = [decps_pool.tile([128, VTILE], f32, tag=f"ps{j}",
                                           name=f"ps{vg}_{t}_{j}")
                           for j in range(G)]
                    for k in range(KT):
                        for j in range(G):
                            nc.tensor.matmul(
                                ps3[j],
                                hnT_t[t][:, k, :],
                                wd3[j][:, k, :],
                                start=(k == 0),
                                stop=(k == KT - 1 and not with_bd),
                            )
                    for j in range(G):
                        v = vg * G + j
                        if with_bd:
                            nc.tensor.matmul(
                                ps3[j], ones_sb,
                                bdT_sb[0:1, v * VTILE:(v + 1) * VTILE],
                                start=False, stop=True,
                            )
                        if DIAG_SKIP_OUT_COPY:
                            continue
                        ot = outp.tile([128, VTILE], f32, tag="ot")
                        if (v * NT + t + j) % 2 == 0:
                            nc.scalar.copy(out=ot, in_=ps3[j])
                        else:
                            nc.vector.tensor_copy(out=ot, in_=ps3[j])
                        if not DIAG_SKIP_OUT_DMA:
                            nc.sync.dma_start(out=out_tiled[t, :, v, :], in_=ot)


def _get_program(T_cap, V_cap, with_bt, with_bd):
    key = (T_cap, V_cap, with_bt, with_bd, DEC_BF16, PRE_ROUND)
    if key not in _prog_cache:
        _prog_cache[key] = _build_program(T_cap, V_cap, with_bt, with_bd,
                                          dec_bf16=DEC_BF16,
                                          pre_rounded=PRE_ROUND)
    return _prog_cache[key]


def kernel(last_hidden_states, cluster_labels, W_t, b_t, ln_gamma, ln_beta,
           dec_weights, dec_biases):
    from concourse import bass_utils

    x = np.ascontiguousarray(np.asarray(last_hidden_states, dtype=np.float32))
    B, S, Hx = x.shape
    assert Hx == H
    labels = np.asarray(cluster_labels).reshape(-1)
    W_t = np.asarray(W_t, dtype=np.float32)
    b_t = np.asarray(b_t, dtype=np.float32)
    gamma = np.asarray(ln_gamma, dtype=np.float32)
    beta = np.asarray(ln_beta, dtype=np.float32)

    x_flat = x.reshape(-1, H)
    idxs = [np.nonzero(labels == i)[0] for i in range(N_CLUSTERS)]
    counts = [len(ix) for ix in idxs]
    T_cap = max(128, ((max(counts) + 127) // 128) * 128)
    V_cap = max(((v + VTILE - 1) // VTILE) * VTILE for v in VOCABS)

    with_bt = bool(np.any(b_t != 0.0))
    wtT = np.ascontiguousarray(W_t.T)

    # fold LN affine into decoder weights/biases:
    # (hn*gamma+beta) @ Wd^T + b == hn @ (Wd*gamma)^T + (b + Wd@beta)
    wdTs, bds = [], []
    for i in range(N_CLUSTERS):
        wd = np.asarray(dec_weights[i], dtype=np.float32)
        bd = np.asarray(dec_biases[i], dtype=np.float32)
        v = wd.shape[0]
        wdT = np.zeros((H, V_cap), dtype=np.float32)
        wdT[:, :v] = (wd * gamma[None, :]).T
        bfold = np.zeros((V_cap,), dtype=np.float32)
        bfold[:v] = bd + wd @ beta
        if DEC_BF16:
            import ml_dtypes
            wdT = wdT.astype(ml_dtypes.bfloat16)
        wdTs.append(wdT)
        bds.append(bfold)
    with_bd = bool(any(np.any(b != 0.0) for b in bds))

    nc = _get_program(T_cap, V_cap, with_bt, with_bd)

    ident = np.eye(128, dtype=np.float32)
    ones1 = np.ones((1, 128), dtype=np.float32)

    if PRE_ROUND:
        wtT = _round_f32r(wtT)
        if not DEC_BF16:
            wdTs = [_round_f32r(w) for w in wdTs]

    in_maps = []
    for i in range(N_CLUSTERS):
        xT = np.zeros((H, T_cap), dtype=np.float32)
        if counts[i]:
            xT[:, :counts[i]] = x_flat[idxs[i]].T
        if PRE_ROUND:
            xT = _round_f32r(xT)
        m = {"xT": xT, "wtT": wtT, "wdT": wdTs[i], "ident": ident, "ones1": ones1}
        if with_bt:
            m["btT"] = _round_f32r(b_t.reshape(1, H)) if PRE_ROUND else b_t.reshape(1, H)
        if with_bd:
            m["bdT"] = _round_f32r(bds[i].reshape(1, V_cap)) if PRE_ROUND else bds[i].reshape(1, V_cap)
        in_maps.append(m)

    res = bass_utils.run_bass_kernel_spmd(nc, in_maps, core_ids=list(range(8)))

    outs = []
    for i in range(N_CLUSTERS):
        v = VOCABS[i]
        full = np.zeros((B * S, v), dtype=np.float32)
        if counts[i]:
            full[idxs[i]] = res.results[i]["out"][:counts[i], :v]
        outs.append(full.reshape(B, S, v))
    return tuple(outs)
